# revision 23
# baseline (speedup 1.0000x reference)
"""Trainium2 Bass kernel for CAM (channel attention module).

Reference computation (per batch b):
    q = x_low[b]  as [C, N]   (C=512, N=64*64=4096)
    k = x_high[b] as [C, N]
    E = q @ k.T                              # [C, C]
    att = softmax(rowmax(E) - E, axis=-1)    # == exp(rowmin(E) - E) / Z
    out = gamma * (att @ k) + x_low[b]
Sharding: data-parallel over batch. 16 batches / 8 cores = 2 per core.

Software-pipelined across the two per-core batches: the PE program
order interleaves batch b's transpose+mm1 chunks with batch b-1's mm2
output tiles so the PE never idles across the softmax/attT batch
boundary, and mm1 matmuls lag their chunk's transposes by one chunk so
the PSUM->SBUF copies of the transposed operands are off the critical
path. Input pools hold 1.25 batches so the load stream rarely stalls
on SBUF slots. Dependency-free identity transposes ("warm" tiles) run
at kernel start and across each softmax boundary to keep the PE HAM
clock-gate at 8/8 (a >~3us PE idle window re-throttles the PE array to
1.2 GHz for several microseconds).

Measured on trn2 (8 cores, per-core trace): 230us baseline -> 207us.
PE-bound: ~184us PE busy (544 transposes @ ~85ns + 512 N=512 f32r
matmuls @ ~228ns back-to-back + LDWEIGHTS mostly hidden), DMA ~50MB
at up to ~390 GB/s. Deeper interleaving of mm2 (lag 1-2 superchunks)
was tried and REGRESSED (~233-243us): it eats the load-prefetch slack
in the kn/qn rings and the Sync ring head-of-line blocks, starving the
transposes. Loads must stay on the Sync ring only: putting them on the
Scalar ring head-of-line blocks the KT PSUM->SBUF copies behind 600ns
dma_start issue slots.
"""

import sys

sys.path.insert(0, "/opt/trn_rl_repo")

import numpy as np

B, C, H, W = 16, 512, 64, 64
N = H * W               # 4096
N_CORES = 8
B_LOC = B // N_CORES    # 2 batches per core
P = 128                 # partitions
CP = C // P             # 4 channel chunks
NP = N // P             # 32 n chunks of 128
FB = 512                # free-dim block (psum bank) for mm2 output
NB = N // FB            # 8 n blocks of 512
ST = 512                # load sub-tile free size
NS = N // ST            # 4 sub-tiles per (tensor, cc)

_CACHE = {}


def _build_module(reps=0):
    import contextlib
    import concourse.bacc as bacc
    import concourse.tile as tile
    import concourse.mybir as mybir
    from concourse.masks import make_identity

    f32 = mybir.dt.float32
    f32r = mybir.dt.float32r

    nc = bacc.Bacc("TRN2", target_bir_lowering=False, debug=False)

    xh = nc.dram_tensor("xh", [B_LOC, C, N], f32r, kind="ExternalInput")
    xl = nc.dram_tensor("xl", [B_LOC, C, N], f32r, kind="ExternalInput")
    gm = nc.dram_tensor("gm", [P, 1], f32, kind="ExternalInput")
    out = nc.dram_tensor("out", [B_LOC, C, N], f32, kind="ExternalOutput")

    def r(ap):
        return ap.bitcast(f32r)

    def rf(ap):
        return ap.bitcast(f32)

    with tile.TileContext(nc) as tc:
        with (
            tc.tile_pool(name="const", bufs=1) as const_pool,
            tc.tile_pool(name="kn", bufs=NS * CP + 8) as kn_pool,
            tc.tile_pool(name="qn", bufs=NS * CP + 8) as qn_pool,
            tc.tile_pool(name="tT", bufs=2) as tT_pool,
            tc.tile_pool(name="soft", bufs=CP) as soft_pool,
            tc.tile_pool(name="attT", bufs=CP) as attT_pool,
            tc.tile_pool(name="osb", bufs=8) as out_pool,
            tc.tile_pool(name="small", bufs=24) as small_pool,
            tc.tile_pool(name="psE", bufs=CP, space="PSUM") as psE_pool,
            tc.tile_pool(name="psW", bufs=4, space="PSUM") as psW_pool,
        ):
            ident_f = const_pool.tile([P, P], f32)
            make_identity(nc, ident_f[:])
            ident = const_pool.tile([P, P], f32r)
            nc.vector.tensor_copy(ident[:], ident_f[:])
            gsb = const_pool.tile([P, 1], f32)
            nc.sync.dma_start(gsb[:], gm.ap())

            def load_batch(b):
                # All loads on the Sync HWDGE ring (it has nothing else to
                # do, so slot-wait head-of-line blocking is harmless there).
                KN = [[None] * NS for _ in range(CP)]
                QN = [[None] * NS for _ in range(CP)]
                for s in range(NS):
                    ssl = slice(s * ST, (s + 1) * ST)
                    for cc in range(CP):
                        csl = slice(cc * P, (cc + 1) * P)
                        kt = kn_pool.tile([P, ST], f32r, tag="kn", name=f"kn{b}_{cc}_{s}")
                        qt = qn_pool.tile([P, ST], f32r, tag="qn", name=f"qn{b}_{cc}_{s}")
                        nc.sync.dma_start(kt[:], xh.ap()[b, csl, ssl])
                        nc.sync.dma_start(qt[:], xl.ap()[b, csl, ssl])
                        KN[cc][s] = kt
                        QN[cc][s] = qt
                return KN, QN

            def blk(TILES, cc, lo, width):
                s = lo // ST
                o = lo - s * ST
                return TILES[cc][s][:, o:o + width]

            def t_stage(st, nn):
                # 8 PE transposes for chunk nn -> qtp/ktp PSUM, then copies
                # to SBUF (DVE for q, ACT for k).
                b = st["b"]
                qtp = psW_pool.tile([P, FB], f32, tag="wp", name=f"qtp{b}_{nn}")
                ktp = psW_pool.tile([P, FB], f32, tag="wp", name=f"ktp{b}_{nn}")
                for cc in range(CP):
                    csl = slice(cc * P, (cc + 1) * P)
                    nc.tensor.transpose(
                        r(qtp[:, csl]), r(blk(st["QN"], cc, nn * P, P)), r(ident[:]))
                    nc.tensor.transpose(
                        r(ktp[:, csl]), r(blk(st["KN"], cc, nn * P, P)), r(ident[:]))
                QT = tT_pool.tile([P, FB], f32r, tag="qt", name=f"QT{b}_{nn}")
                nc.vector.tensor_copy(QT[:], qtp[:])
                KT = tT_pool.tile([P, FB], f32r, tag="kt", name=f"KT{b}_{nn}")
                nc.scalar.copy(KT[:], ktp[:])
                st["QT"][nn] = QT
                st["KT"][nn] = KT

            def m1_stage(st, nn):
                QT = st["QT"].pop(nn)
                KT = st["KT"].pop(nn)
                for ic in range(CP):
                    nc.tensor.matmul(
                        st["E"][ic][:],
                        r(QT[:, ic * P:(ic + 1) * P]),
                        r(KT[:]),
                        start=(nn == 0),
                        stop=(nn == NP - 1),
                    )

            def softmax_stage(st):
                # att = gamma * exp(m - E) / Z   (m = rowmin)
                b = st["b"]
                att = []
                for ic in range(CP):
                    m = small_pool.tile([P, 1], f32, tag="m")
                    nc.vector.tensor_reduce(
                        m[:], st["E"][ic][:], axis=mybir.AxisListType.X,
                        op=mybir.AluOpType.min,
                    )
                    a = soft_pool.tile([P, FB], f32r, tag="att", name=f"att{b}_{ic}")
                    z = small_pool.tile([P, 1], f32, tag="z")
                    nc.scalar.activation(
                        a[:], st["E"][ic][:], mybir.ActivationFunctionType.Exp,
                        bias=m[:], scale=-1.0, accum_out=z[:],
                    )
                    zinv = small_pool.tile([P, 1], f32, tag="zi")
                    nc.vector.reciprocal(zinv[:], z[:])
                    asc = small_pool.tile([P, 1], f32, tag="as")
                    nc.vector.tensor_mul(asc[:], zinv[:], gsb[:])
                    nc.vector.tensor_scalar_mul(a[:], a[:], asc[:])
                    att.append(a)
                st["att"] = att

            def attT_stage(st):
                b = st["b"]
                attT = []
                for jc in range(CP):
                    atp = psW_pool.tile([P, FB], f32, tag="wp", name=f"atp{b}_{jc}")
                    jsl = slice(jc * P, (jc + 1) * P)
                    for ic in range(CP):
                        nc.tensor.transpose(
                            r(atp[:, ic * P:(ic + 1) * P]),
                            r(st["att"][ic][:, jsl]), r(ident[:]),
                        )
                    aT = attT_pool.tile([P, FB], f32r, tag="attT", name=f"aT{b}_{jc}")
                    if jc % 2 == 0:
                        nc.vector.tensor_copy(aT[:], atp[:])
                    else:
                        nc.scalar.copy(aT[:], atp[:])
                    attT.append(aT)
                st["attT"] = attT

            def o_stage(st, j):
                # one mm2 output tile: 4 accumulating matmuls + residual + store
                b = st["b"]
                nb, ic = j // CP, j % CP
                isl = slice(ic * P, (ic + 1) * P)
                ops = psW_pool.tile([P, FB], f32, tag="wp", name=f"ops{b}_{nb}_{ic}")
                for jc in range(CP):
                    nc.tensor.matmul(
                        ops[:],
                        r(st["attT"][jc][:, isl]),
                        r(blk(st["KN"], jc, nb * FB, FB)),
                        start=(jc == 0),
                        stop=(jc == CP - 1),
                    )
                osb = out_pool.tile([P, FB], f32, tag="osb")
                nc.vector.tensor_add(osb[:], ops[:], rf(blk(st["QN"], ic, nb * FB, FB)))
                nc.scalar.dma_start(out.ap()[b, isl, nb * FB:(nb + 1) * FB], osb[:])

            rep_ctx = tc.For_i(0, reps, 1) if reps else contextlib.nullcontext()
            with rep_ctx:
                states = []
                for b in range(B_LOC):
                    KN, QN = load_batch(b)
                    states.append({
                        "b": b, "KN": KN, "QN": QN,
                        "QT": {}, "KT": {},
                        "E": None,
                    })
                warm_n = [0]

                def warm(count):
                    # Dependency-free PE work: keeps the PE streaming (and
                    # HAM un-throttled) across waits it would otherwise
                    # idle through.
                    for _ in range(count):
                        w = warm_n[0]
                        warm_n[0] += 1
                        wp = psW_pool.tile([P, P], f32, tag="wp",
                                           name=f"warm{w}", padded_shape=[P, FB])
                        nc.tensor.transpose(r(wp[:]), r(ident[:]), r(ident[:]))

                warm(24)
                prev = None
                for b in range(B_LOC):
                    st = states[b]
                    st["E"] = [
                        psE_pool.tile([P, FB], f32, tag="E", name=f"E{b}_{i}")
                        for i in range(CP)
                    ]
                    for nn in range(0, NP, 2):
                        # Two chunks per superchunk: halves the number of
                        # transpose-mode <-> matmul-mode PE switches, each
                        # of which costs ~150ns of pipeline refill.
                        t_stage(st, nn)
                        t_stage(st, nn + 1)
                        if nn > 0:
                            m1_stage(st, nn - 2)
                            m1_stage(st, nn - 1)
                        if prev is not None:
                            o_stage(prev, nn)
                            o_stage(prev, nn + 1)
                    m1_stage(st, NP - 2)
                    m1_stage(st, NP - 1)
                    softmax_stage(st)
                    warm(24)
                    attT_stage(st)
                    prev = st
                for j in range(NB * CP):
                    o_stage(prev, j)

    nc.compile()
    return nc


def _build(reps=0, **kw):
    return _build_module(reps=reps)


def _get_module():
    if "nc" not in _CACHE:
        _CACHE["nc"] = _build()
    return _CACHE["nc"]


def kernel(x_high, x_low, gamma):
    from concourse.bass_utils import run_bass_kernel_spmd

    nc = _get_module()

    x_high = np.ascontiguousarray(np.asarray(x_high), dtype=np.float32)
    x_low = np.ascontiguousarray(np.asarray(x_low), dtype=np.float32)
    gamma = np.asarray(gamma, dtype=np.float32).reshape(-1)

    xh3 = x_high.reshape(B, C, N)
    xl3 = x_low.reshape(B, C, N)
    gm = np.full((P, 1), gamma[0], dtype=np.float32)

    in_maps = []
    for i in range(N_CORES):
        sl = slice(i * B_LOC, (i + 1) * B_LOC)
        in_maps.append({
            "xh": np.ascontiguousarray(xh3[sl]),
            "xl": np.ascontiguousarray(xl3[sl]),
            "gm": gm,
        })

    res = run_bass_kernel_spmd(nc, in_maps, list(range(N_CORES)))
    out = np.concatenate([res.results[i]["out"] for i in range(N_CORES)], axis=0)
    return out.reshape(B, C, H, W)


# revision 31
# speedup vs baseline: 1.0322x; 1.0322x over previous
"""Trainium2 Bass kernel for CAM (channel attention module).

Reference computation (per batch b):
    q = x_low[b]  as [C, N]   (C=512, N=64*64=4096)
    k = x_high[b] as [C, N]
    E = q @ k.T                              # [C, C]
    att = softmax(rowmax(E) - E, axis=-1)    # == exp(rowmin(E) - E) / Z
    out = gamma * (att @ k) + x_low[b]
Sharding: data-parallel over batch. 16 batches / 8 cores = 2 per core.

Software-pipelined across the two per-core batches: the PE program
order interleaves batch b's transpose+mm1 chunks with batch b-1's mm2
output tiles so the PE never idles across the softmax/attT batch
boundary, and mm1 matmuls lag their chunk's transposes by one chunk so
the PSUM->SBUF copies of the transposed operands are off the critical
path. Input pools hold 1.25 batches so the load stream rarely stalls
on SBUF slots. Dependency-free identity transposes ("warm" tiles) run
at kernel start and across each softmax boundary to keep the PE HAM
clock-gate at 8/8 (a >~3us PE idle window re-throttles the PE array to
1.2 GHz for several microseconds).

Measured on trn2 (8 cores, per-core trace): 230us baseline -> 207us.
PE-bound: ~184us PE busy (544 transposes @ ~85ns + 512 N=512 f32r
matmuls @ ~228ns back-to-back + LDWEIGHTS mostly hidden), DMA ~50MB
at up to ~390 GB/s. Deeper interleaving of mm2 (lag 1-2 superchunks)
was tried and REGRESSED (~233-243us): it eats the load-prefetch slack
in the kn/qn rings and the Sync ring head-of-line blocks, starving the
transposes. Loads must stay on the Sync ring only: putting them on the
Scalar ring head-of-line blocks the KT PSUM->SBUF copies behind 600ns
dma_start issue slots.
"""

import sys

sys.path.insert(0, "/opt/trn_rl_repo")

import numpy as np

B, C, H, W = 16, 512, 64, 64
N = H * W               # 4096
N_CORES = 8
B_LOC = B // N_CORES    # 2 batches per core
P = 128                 # partitions
CP = C // P             # 4 channel chunks
NP = N // P             # 32 n chunks of 128
FB = 512                # free-dim block (psum bank) for mm2 output
NB = N // FB            # 8 n blocks of 512
ST = 512                # load sub-tile free size
NS = N // ST            # 4 sub-tiles per (tensor, cc)

_CACHE = {}


def _build_module(reps=0):
    import contextlib
    import concourse.bacc as bacc
    import concourse.tile as tile
    import concourse.mybir as mybir
    from concourse.masks import make_identity

    f32 = mybir.dt.float32
    f32r = mybir.dt.float32r

    nc = bacc.Bacc("TRN2", target_bir_lowering=False, debug=False)

    xh = nc.dram_tensor("xh", [B_LOC, C, N], f32r, kind="ExternalInput")
    xl = nc.dram_tensor("xl", [B_LOC, C, N], f32r, kind="ExternalInput")
    gm = nc.dram_tensor("gm", [P, 1], f32, kind="ExternalInput")
    out = nc.dram_tensor("out", [B_LOC, C, N], f32, kind="ExternalOutput")

    def r(ap):
        return ap.bitcast(f32r)

    def rf(ap):
        return ap.bitcast(f32)

    with tile.TileContext(nc) as tc:
        with (
            tc.tile_pool(name="const", bufs=1) as const_pool,
            tc.tile_pool(name="kn", bufs=NS * CP + 8) as kn_pool,
            tc.tile_pool(name="qn", bufs=NS * CP + 8) as qn_pool,
            tc.tile_pool(name="tT", bufs=3) as tT_pool,
            tc.tile_pool(name="soft", bufs=CP) as soft_pool,
            tc.tile_pool(name="attT", bufs=CP) as attT_pool,
            tc.tile_pool(name="osb", bufs=8) as out_pool,
            tc.tile_pool(name="small", bufs=16) as small_pool,
            tc.tile_pool(name="psE", bufs=CP, space="PSUM") as psE_pool,
            tc.tile_pool(name="psW", bufs=4, space="PSUM") as psW_pool,
        ):
            ident_f = const_pool.tile([P, P], f32)
            make_identity(nc, ident_f[:])
            ident = const_pool.tile([P, P], f32r)
            nc.vector.tensor_copy(ident[:], ident_f[:])
            gsb = const_pool.tile([P, 1], f32)
            nc.sync.dma_start(gsb[:], gm.ap())

            def load_batch(b):
                # All loads on the Sync HWDGE ring (it has nothing else to
                # do, so slot-wait head-of-line blocking is harmless there).
                KN = [[None] * NS for _ in range(CP)]
                QN = [[None] * NS for _ in range(CP)]
                for s in range(NS):
                    ssl = slice(s * ST, (s + 1) * ST)
                    for cc in range(CP):
                        csl = slice(cc * P, (cc + 1) * P)
                        kt = kn_pool.tile([P, ST], f32r, tag="kn", name=f"kn{b}_{cc}_{s}")
                        qt = qn_pool.tile([P, ST], f32r, tag="qn", name=f"qn{b}_{cc}_{s}")
                        nc.sync.dma_start(kt[:], xh.ap()[b, csl, ssl])
                        nc.sync.dma_start(qt[:], xl.ap()[b, csl, ssl])
                        KN[cc][s] = kt
                        QN[cc][s] = qt
                return KN, QN

            def blk(TILES, cc, lo, width):
                s = lo // ST
                o = lo - s * ST
                return TILES[cc][s][:, o:o + width]

            def t_stage(st, nn):
                # 8 PE transposes for chunk nn -> qtp/ktp PSUM, then copies
                # to SBUF (DVE for q, ACT for k).
                b = st["b"]
                qtp = psW_pool.tile([P, FB], f32, tag="wp", name=f"qtp{b}_{nn}")
                ktp = psW_pool.tile([P, FB], f32, tag="wp", name=f"ktp{b}_{nn}")
                for cc in range(CP):
                    csl = slice(cc * P, (cc + 1) * P)
                    nc.tensor.transpose(
                        r(qtp[:, csl]), r(blk(st["QN"], cc, nn * P, P)), r(ident[:]))
                    nc.tensor.transpose(
                        r(ktp[:, csl]), r(blk(st["KN"], cc, nn * P, P)), r(ident[:]))
                QT = tT_pool.tile([P, FB], f32r, tag="qt", name=f"QT{b}_{nn}")
                nc.vector.tensor_copy(QT[:], qtp[:])
                KT = tT_pool.tile([P, FB], f32r, tag="kt", name=f"KT{b}_{nn}")
                nc.scalar.copy(KT[:], ktp[:])
                st["QT"][nn] = QT
                st["KT"][nn] = KT

            def m1_stage(st, nn):
                QT = st["QT"].pop(nn)
                KT = st["KT"].pop(nn)
                for ic in range(CP):
                    nc.tensor.matmul(
                        st["E"][ic][:],
                        r(QT[:, ic * P:(ic + 1) * P]),
                        r(KT[:]),
                        start=(nn == 0),
                        stop=(nn == NP - 1),
                    )

            def softmax_stage(st):
                # att = gamma * exp(m - E) / Z   (m = rowmin)
                b = st["b"]
                att = []
                for ic in range(CP):
                    m = small_pool.tile([P, 1], f32, tag="m")
                    nc.vector.tensor_reduce(
                        m[:], st["E"][ic][:], axis=mybir.AxisListType.X,
                        op=mybir.AluOpType.min,
                    )
                    a = soft_pool.tile([P, FB], f32r, tag="att", name=f"att{b}_{ic}")
                    z = small_pool.tile([P, 1], f32, tag="z")
                    nc.scalar.activation(
                        a[:], st["E"][ic][:], mybir.ActivationFunctionType.Exp,
                        bias=m[:], scale=-1.0, accum_out=z[:],
                    )
                    zinv = small_pool.tile([P, 1], f32, tag="zi")
                    nc.vector.reciprocal(zinv[:], z[:])
                    asc = small_pool.tile([P, 1], f32, tag="as")
                    nc.vector.tensor_mul(asc[:], zinv[:], gsb[:])
                    nc.vector.tensor_scalar_mul(a[:], a[:], asc[:])
                    att.append(a)
                st["att"] = att

            def attT_stage(st):
                b = st["b"]
                attT = []
                for jc in range(CP):
                    atp = psW_pool.tile([P, FB], f32, tag="wp", name=f"atp{b}_{jc}")
                    jsl = slice(jc * P, (jc + 1) * P)
                    for ic in range(CP):
                        nc.tensor.transpose(
                            r(atp[:, ic * P:(ic + 1) * P]),
                            r(st["att"][ic][:, jsl]), r(ident[:]),
                        )
                    aT = attT_pool.tile([P, FB], f32r, tag="attT", name=f"aT{b}_{jc}")
                    if jc % 2 == 0:
                        nc.vector.tensor_copy(aT[:], atp[:])
                    else:
                        nc.scalar.copy(aT[:], atp[:])
                    attT.append(aT)
                st["attT"] = attT

            def o_stage(st, j):
                # one mm2 output tile: 4 accumulating matmuls + residual + store
                b = st["b"]
                nb, ic = j // CP, j % CP
                isl = slice(ic * P, (ic + 1) * P)
                ops = psW_pool.tile([P, FB], f32, tag="wp", name=f"ops{b}_{nb}_{ic}")
                for jc in range(CP):
                    nc.tensor.matmul(
                        ops[:],
                        r(st["attT"][jc][:, isl]),
                        r(blk(st["KN"], jc, nb * FB, FB)),
                        start=(jc == 0),
                        stop=(jc == CP - 1),
                    )
                osb = out_pool.tile([P, FB], f32, tag="osb")
                nc.vector.tensor_add(osb[:], ops[:], rf(blk(st["QN"], ic, nb * FB, FB)))
                nc.scalar.dma_start(out.ap()[b, isl, nb * FB:(nb + 1) * FB], osb[:])

            rep_ctx = tc.For_i(0, reps, 1) if reps else contextlib.nullcontext()
            with rep_ctx:
                states = []
                for b in range(B_LOC):
                    KN, QN = load_batch(b)
                    states.append({
                        "b": b, "KN": KN, "QN": QN,
                        "QT": {}, "KT": {},
                        "E": None,
                    })
                warm_n = [0]

                def warm(count):
                    # Dependency-free PE work: keeps the PE streaming (and
                    # HAM un-throttled) across waits it would otherwise
                    # idle through.
                    for _ in range(count):
                        w = warm_n[0]
                        warm_n[0] += 1
                        wp = psW_pool.tile([P, P], f32, tag="wp",
                                           name=f"warm{w}", padded_shape=[P, FB])
                        nc.tensor.transpose(r(wp[:]), r(ident[:]), r(ident[:]))

                warm(24)
                prev = None
                for b in range(B_LOC):
                    st = states[b]
                    st["E"] = [
                        psE_pool.tile([P, FB], f32, tag="E", name=f"E{b}_{i}")
                        for i in range(CP)
                    ]
                    for nn in range(0, NP, 2):
                        # Two chunks per superchunk halves the ~150ns PE
                        # transpose<->matmul mode-switch cost; tT bufs=3
                        # keeps the lag-2 QT/KT copies off the ACT/DVE
                        # critical path.
                        t_stage(st, nn)
                        t_stage(st, nn + 1)
                        if nn > 0:
                            m1_stage(st, nn - 2)
                            m1_stage(st, nn - 1)
                        if prev is not None:
                            o_stage(prev, nn)
                            o_stage(prev, nn + 1)
                    m1_stage(st, NP - 2)
                    m1_stage(st, NP - 1)
                    softmax_stage(st)
                    warm(24)
                    attT_stage(st)
                    prev = st
                for j in range(NB * CP):
                    o_stage(prev, j)

    nc.compile()
    return nc


def _build(reps=0, **kw):
    return _build_module(reps=reps)


def _get_module():
    if "nc" not in _CACHE:
        _CACHE["nc"] = _build()
    return _CACHE["nc"]


def kernel(x_high, x_low, gamma):
    from concourse.bass_utils import run_bass_kernel_spmd

    nc = _get_module()

    x_high = np.ascontiguousarray(np.asarray(x_high), dtype=np.float32)
    x_low = np.ascontiguousarray(np.asarray(x_low), dtype=np.float32)
    gamma = np.asarray(gamma, dtype=np.float32).reshape(-1)

    xh3 = x_high.reshape(B, C, N)
    xl3 = x_low.reshape(B, C, N)
    gm = np.full((P, 1), gamma[0], dtype=np.float32)

    in_maps = []
    for i in range(N_CORES):
        sl = slice(i * B_LOC, (i + 1) * B_LOC)
        in_maps.append({
            "xh": np.ascontiguousarray(xh3[sl]),
            "xl": np.ascontiguousarray(xl3[sl]),
            "gm": gm,
        })

    res = run_bass_kernel_spmd(nc, in_maps, list(range(N_CORES)))
    out = np.concatenate([res.results[i]["out"] for i in range(N_CORES)], axis=0)
    return out.reshape(B, C, H, W)
